# revision 3
# baseline (speedup 1.0000x reference)
"""Trainium2 Bass kernel for the MINE-style segment_reduce problem.

Computes, for the fixed problem size B=16384, L=512, HID=768, TRANS=128:

    mask   = target.astype(f32)                     # [B, L] of {0,1}
    counts = max(mask.sum(1), 1)
    lf     = (mask @ label_embed) / counts          # [B, HID]
    net(t) = MLP(concat(t @ W_text.T + b_text, lf @ W_label.T + b_label))
    out    = mean(softplus(net(text[perm]))) + mean(softplus(-net(text)))

Key algebraic folding (exact in real arithmetic): the first two linear
layers collapse into

    h1 = relu(text @ A_t.T + (mask @ LW2) / counts + c0)
    A_t = W0[:, :T] @ W_text            # [T, HID]
    LW2 = (label_embed @ W_label.T) @ W0[:, T:].T   # [L, T]
    c0  = b0 + W0[:, :T] @ b_text + W0[:, T:] @ b_label

so label_embed never needs to reach the device, and the per-sample
network is two small matmuls + relu + softplus.

Sharding: data-parallel over B across 8 NeuronCores (2048 rows each).
negative_text = text[perm] is realized host-side as a per-shard gather of
the (transposed, bf16-cast) text matrix. Each core returns the partial sum
of softplus terms over its rows; the host adds 8 scalars and divides by B.

Device layout is feature-major ("T layout", batch on the free dimension),
prepared host-side, so every matmul streams the batch through a stationary
weight tile. All heavy operands are bf16 (the 0/1 mask is exact in bf16);
accumulation is f32 in PSUM.
"""

import numpy as np
import ml_dtypes

B, L, HID, TRANS = 16384, 512, 768, 128
NCORES = 8
BS = B // NCORES          # 2048 rows per core
BT = 512                  # batch tile (free-dim columns per PSUM bank)
NT = BS // BT             # 4 tiles per core
HC = HID // 128           # 6 contraction chunks for text
LC = L // 128             # 4 contraction chunks for the mask

BF16 = ml_dtypes.bfloat16

_CACHE = {}


def _split_sync_waits(nc, mybir, maxw=1):
    """Walrus in this container rejects >1 sync-wait per instruction
    ("Too many sync wait commands"). Hoist excess waits onto NoOps that
    precede the instruction on the same engine."""
    for f in nc.m.functions:
        for bb in f.blocks:
            new = []
            for inst in bb.instructions:
                si = inst.sync_info
                if si is not None and si.on_wait is not None and len(si.on_wait) > maxw:
                    waits = list(si.on_wait)
                    head, rest = waits[:-maxw], waits[-maxw:]
                    for k in range(0, len(head), maxw):
                        nop = mybir.InstNoOp(name=f"{inst.name}-w{k}", ins=[], outs=[])
                        nop.engine = inst.engine
                        nop.sync_info = mybir.SyncInfo(
                            on_wait=head[k : k + maxw], on_update=[]
                        )
                        new.append(nop)
                    inst.sync_info = mybir.SyncInfo(
                        on_wait=rest, on_update=list(si.on_update or [])
                    )
                new.append(inst)
            bb.instructions = new


def _build():
    import concourse.bass as bass
    import concourse.mybir as mybir
    import concourse.tile as tile

    f32 = mybir.dt.float32
    bf16 = mybir.dt.bfloat16

    nc = bass.Bass("TRN2", target_bir_lowering=False, debug=False, num_devices=NCORES)

    xt_d = nc.declare_dram_parameter("xt", [HID, BS], bf16, isOutput=False)
    xn_d = nc.declare_dram_parameter("xn", [HID, BS], bf16, isOutput=False)
    mt_d = nc.declare_dram_parameter("mt", [L, BS], bf16, isOutput=False)
    cinv_d = nc.declare_dram_parameter("cinv", [1, BS], f32, isOutput=False)
    atT_d = nc.declare_dram_parameter("atT", [HID, TRANS], bf16, isOutput=False)
    lw2_d = nc.declare_dram_parameter("lw2", [L, TRANS], bf16, isOutput=False)
    w1T_d = nc.declare_dram_parameter("w1T", [TRANS, TRANS], bf16, isOutput=False)
    w2T_d = nc.declare_dram_parameter("w2T", [TRANS, 1], bf16, isOutput=False)
    ones_d = nc.declare_dram_parameter("ones", [1, TRANS], f32, isOutput=False)
    c0_d = nc.declare_dram_parameter("c0", [TRANS, 1], f32, isOutput=False)
    b1_d = nc.declare_dram_parameter("b1v", [TRANS, 1], f32, isOutput=False)
    b2_d = nc.declare_dram_parameter("b2v", [1, 2], f32, isOutput=False)
    out_d = nc.declare_dram_parameter("out", [1, 1], f32, isOutput=True)

    AF = mybir.ActivationFunctionType
    ALU = mybir.AluOpType

    with tile.TileContext(nc) as tc:
        with (
            tc.tile_pool(name="const", bufs=1) as cpool,
            tc.tile_pool(name="xload", bufs=3) as xpool,
            tc.tile_pool(name="mload", bufs=2) as mpool,
            tc.tile_pool(name="work", bufs=2) as wpool,
            tc.tile_pool(name="psum_uv", bufs=2, space="PSUM") as puv,
            tc.tile_pool(name="psum_v", bufs=2, space="PSUM") as pv,
            tc.tile_pool(name="psum_h2", bufs=2, space="PSUM") as ph2,
            tc.tile_pool(name="psum_e", bufs=2, space="PSUM") as pe,
        ):
            # ---- constants / small operands ----
            atT_sb = cpool.tile([128, HC, TRANS], bf16, tag="atT")
            nc.sync.dma_start(
                atT_sb[:], atT_d.ap().rearrange("(c p) m -> p c m", p=128)
            )
            lw2_sb = cpool.tile([128, LC, TRANS], bf16, tag="lw2")
            nc.sync.dma_start(
                lw2_sb[:], lw2_d.ap().rearrange("(c p) m -> p c m", p=128)
            )
            w1T_sb = cpool.tile([128, TRANS], bf16, tag="w1T")
            nc.sync.dma_start(w1T_sb[:], w1T_d[:, :])
            w2T_sb = cpool.tile([128, 1], bf16, tag="w2T")
            nc.sync.dma_start(w2T_sb[:], w2T_d[:, :])
            ones_sb = cpool.tile([1, TRANS], f32, tag="ones")
            nc.sync.dma_start(ones_sb[:], ones_d[:, :])
            c0_sb = cpool.tile([TRANS, 1], f32, tag="c0")
            nc.sync.dma_start(c0_sb[:], c0_d[:, :])
            b1_sb = cpool.tile([TRANS, 1], f32, tag="b1")
            nc.sync.dma_start(b1_sb[:], b1_d[:, :])
            b2_sb = cpool.tile([1, 2], f32, tag="b2")
            nc.sync.dma_start(b2_sb[:], b2_d[:, :])
            cinv_sb = cpool.tile([1, BS], f32, tag="cinv")
            nc.sync.dma_start(cinv_sb[:], cinv_d[:, :])

            # per-(tile, stream) softplus partial sums
            acc_sb = cpool.tile([1, 2 * NT], f32, tag="acc")
            nc.vector.memset(acc_sb[:, :], 0.0)

            # ---- broadcast 1/counts across partitions: cb = ones.T @ cinv ----
            cb_sb = cpool.tile([128, BS], f32, tag="cb")
            for i in range(NT):
                cb_ps = puv.tile([128, BT], f32, tag="u")
                nc.tensor.matmul(
                    cb_ps[:, :],
                    ones_sb[:, :],
                    cinv_sb[:, i * BT : (i + 1) * BT],
                    start=True,
                    stop=True,
                )
                nc.vector.tensor_copy(cb_sb[:, i * BT : (i + 1) * BT], cb_ps[:, :])

            # ---- main loop over batch tiles ----
            for i in range(NT):
                sl = slice(i * BT, (i + 1) * BT)

                mt_i = mpool.tile([128, LC, BT], bf16, tag="mt")
                nc.sync.dma_start(
                    mt_i[:], mt_d[:, sl].rearrange("(c p) n -> p c n", p=128)
                )
                xt_i = xpool.tile([128, HC, BT], bf16, tag="xt")
                nc.sync.dma_start(
                    xt_i[:], xt_d[:, sl].rearrange("(c p) n -> p c n", p=128)
                )
                xn_i = xpool.tile([128, HC, BT], bf16, tag="xn")
                nc.sync.dma_start(
                    xn_i[:], xn_d[:, sl].rearrange("(c p) n -> p c n", p=128)
                )

                # v = (mask @ LW2).T for this tile, then vs = v / counts
                v_ps = pv.tile([128, BT], f32, tag="v")
                for c in range(LC):
                    nc.tensor.matmul(
                        v_ps[:, :],
                        lw2_sb[:, c, :],
                        mt_i[:, c, :],
                        start=(c == 0),
                        stop=(c == LC - 1),
                    )
                vs_sb = wpool.tile([128, BT], f32, tag="vs")
                nc.vector.tensor_mul(vs_sb[:, :], v_ps[:, :], cb_sb[:, sl])

                for s, x_i in enumerate((xt_i, xn_i)):
                    u_ps = puv.tile([128, BT], f32, tag="u")
                    for c in range(HC):
                        nc.tensor.matmul(
                            u_ps[:, :],
                            atT_sb[:, c, :],
                            x_i[:, c, :],
                            start=(c == 0),
                            stop=(c == HC - 1),
                        )
                    # t = u + c0 + vs in one DVE op, then relu -> bf16 on ACT
                    t_sb = wpool.tile([128, BT], f32, tag="t")
                    nc.vector.scalar_tensor_tensor(
                        t_sb[:, :],
                        u_ps[:, :],
                        c0_sb[:, :],
                        vs_sb[:, :],
                        op0=ALU.add,
                        op1=ALU.add,
                    )
                    h1_sb = wpool.tile([128, BT], bf16, tag="h1")
                    nc.scalar.activation(h1_sb[:, :], t_sb[:, :], AF.Relu)

                    h2_ps = ph2.tile([128, BT], f32, tag="h2")
                    nc.tensor.matmul(
                        h2_ps[:, :], w1T_sb[:, :], h1_sb[:, :], start=True, stop=True
                    )
                    h2_sb = wpool.tile([128, BT], bf16, tag="h2s")
                    nc.scalar.activation(h2_sb[:, :], h2_ps[:, :], AF.Relu, bias=b1_sb[:, :])

                    e_ps = pe.tile([1, BT], f32, tag="e")
                    nc.tensor.matmul(
                        e_ps[:, :], w2T_sb[:, :], h2_sb[:, :], start=True, stop=True
                    )
                    # joint stream (s==0): softplus(-(e+b2)); marginal: softplus(e+b2)
                    # softplus(x) = ln(1 + exp(x)) — this walrus ACT table has
                    # exp+ln+relu in one func set but no native softplus.
                    # |e+b2| stays O(1) here so exp cannot overflow.
                    sgn = -1.0 if s == 0 else 1.0
                    k = 2 * i + s
                    ex_sb = wpool.tile([1, BT], f32, tag="ex")
                    nc.scalar.activation(
                        ex_sb[:, :],
                        e_ps[:, :],
                        AF.Exp,
                        bias=b2_sb[:, s : s + 1],
                        scale=sgn,
                    )
                    sp_sb = wpool.tile([1, BT], f32, tag="sp")
                    nc.scalar.activation(
                        sp_sb[:, :],
                        ex_sb[:, :],
                        AF.Ln,
                        bias=ones_sb[:, 0:1],
                        accum_out=acc_sb[:, k : k + 1],
                    )

            res_sb = cpool.tile([1, 1], f32, tag="res")
            nc.vector.reduce_sum(res_sb[:, :], acc_sb[:, :], axis=mybir.AxisListType.X)
            nc.sync.dma_start(out_d[:, :], res_sb[:, :])

    _split_sync_waits(nc, mybir, maxw=1)
    return nc


def _get_nc():
    if "nc" not in _CACHE:
        _CACHE["nc"] = _build()
    return _CACHE["nc"]


def _prep_inputs(text_embed, label_embed, target, perm,
                 W_text, b_text, W_label, b_label, W0, b0, W1, b1, W2, b2):
    f64 = np.float64
    W0t = W0[:, :TRANS].astype(f64)
    W0l = W0[:, TRANS:].astype(f64)
    A_t = W0t @ W_text.astype(f64)                                   # [T, HID]
    LW2 = (label_embed.astype(f64) @ W_label.T.astype(f64)) @ W0l.T  # [L, T]
    c0 = b0.astype(f64) + W0t @ b_text.astype(f64) + W0l @ b_label.astype(f64)

    atT = np.ascontiguousarray(A_t.T).astype(BF16)                   # [HID, T]
    lw2 = np.ascontiguousarray(LW2).astype(BF16)                     # [L, T]
    w1T = np.ascontiguousarray(W1.T).astype(BF16)                    # [T, T]
    w2T = np.ascontiguousarray(W2.T).astype(BF16)                    # [T, 1]
    c0v = c0.astype(np.float32).reshape(TRANS, 1)
    b1v = b1.astype(np.float32).reshape(TRANS, 1)
    ones = np.ones((1, TRANS), np.float32)
    b2val = float(np.asarray(b2).reshape(-1)[0])

    counts = np.maximum(target.sum(axis=1), 1).astype(np.float64)
    cinv = (1.0 / counts).astype(np.float32)                         # [B]

    text_T = np.ascontiguousarray(text_embed.T).astype(BF16)         # [HID, B]
    mask_T = np.ascontiguousarray(target.T.astype(np.float32)).astype(BF16)  # [L, B]
    perm = np.asarray(perm).astype(np.int64)

    in_maps = []
    for k in range(NCORES):
        sl = slice(k * BS, (k + 1) * BS)
        in_maps.append({
            "xt": np.ascontiguousarray(text_T[:, sl]),
            "xn": np.ascontiguousarray(text_T[:, perm[sl]]),
            "mt": np.ascontiguousarray(mask_T[:, sl]),
            "cinv": np.ascontiguousarray(cinv[sl]).reshape(1, BS),
            "atT": atT, "lw2": lw2, "w1T": w1T, "w2T": w2T,
            "ones": ones, "c0": c0v, "b1v": b1v,
            "b2v": np.array([[-b2val, b2val]], np.float32),
        })
    return in_maps, b2val


def _run(in_maps, b2val, trace=False):
    from concourse.bass_utils import run_bass_kernel_spmd

    nc = _get_nc()
    res = run_bass_kernel_spmd(nc, in_maps, list(range(NCORES)), trace=trace)
    total = sum(float(res.results[k]["out"][0, 0]) for k in range(NCORES))
    return np.float32(total / B), res


def kernel(text_embed, label_embed, target, perm,
           W_text, b_text, W_label, b_label, W0, b0, W1, b1, W2, b2):
    in_maps, b2val = _prep_inputs(
        text_embed, label_embed, target, perm,
        W_text, b_text, W_label, b_label, W0, b0, W1, b1, W2, b2)
    out, _ = _run(in_maps, b2val)
    return out


# revision 4
# speedup vs baseline: 1.0815x; 1.0815x over previous
"""Trainium2 Bass kernel for the MINE-style segment_reduce problem.

Computes, for the fixed problem size B=16384, L=512, HID=768, TRANS=128:

    mask   = target.astype(f32)                     # [B, L] of {0,1}
    counts = max(mask.sum(1), 1)
    lf     = (mask @ label_embed) / counts          # [B, HID]
    net(t) = MLP(concat(t @ W_text.T + b_text, lf @ W_label.T + b_label))
    out    = mean(softplus(net(text[perm]))) + mean(softplus(-net(text)))

Key algebraic folding (exact in real arithmetic): the first two linear
layers collapse into

    h1 = relu(text @ A_t.T + (mask @ LW2) / counts + c0)
    A_t = W0[:, :T] @ W_text            # [T, HID]
    LW2 = (label_embed @ W_label.T) @ W0[:, T:].T   # [L, T]
    c0  = b0 + W0[:, :T] @ b_text + W0[:, T:] @ b_label

so label_embed never needs to reach the device, and the per-sample
network is two small matmuls + relu + softplus.

Sharding: data-parallel over B across 8 NeuronCores (2048 rows each).
negative_text = text[perm] is realized host-side as a per-shard gather of
the (transposed, bf16-cast) text matrix. Each core returns the partial sum
of softplus terms over its rows; the host adds 8 scalars and divides by B.

Device layout is feature-major ("T layout", batch on the free dimension),
prepared host-side, so every matmul streams the batch through a stationary
weight tile. All heavy operands are bf16 (the 0/1 mask is exact in bf16);
accumulation is f32 in PSUM.
"""

import numpy as np
import ml_dtypes

B, L, HID, TRANS = 16384, 512, 768, 128
NCORES = 8
BS = B // NCORES          # 2048 rows per core
BT = 512                  # batch tile (free-dim columns per PSUM bank)
NT = BS // BT             # 4 tiles per core
HC = HID // 128           # 6 contraction chunks for text
LC = L // 128             # 4 contraction chunks for the mask

BF16 = ml_dtypes.bfloat16

_CACHE = {}


def _split_sync_waits(nc, mybir, maxw_default=1, maxw_drain=1):
    """Walrus in this container rejects too many sync-waits per instruction
    ("Too many sync wait commands"). Hoist excess waits onto NoOps that
    precede the instruction on the same engine."""
    for f in nc.m.functions:
        for bb in f.blocks:
            new = []
            for inst in bb.instructions:
                maxw = maxw_drain if type(inst).__name__ in ("InstDrain", "InstNoOp") else maxw_default
                si = inst.sync_info
                if si is not None and si.on_wait is not None and len(si.on_wait) > maxw:
                    waits = list(si.on_wait)
                    head, rest = waits[:-maxw], waits[-maxw:]
                    for k in range(0, len(head), maxw_drain):
                        nop = mybir.InstNoOp(name=f"{inst.name}-w{k}", ins=[], outs=[])
                        nop.engine = inst.engine
                        nop.sync_info = mybir.SyncInfo(
                            on_wait=head[k : k + maxw_drain], on_update=[]
                        )
                        new.append(nop)
                    inst.sync_info = mybir.SyncInfo(
                        on_wait=rest, on_update=list(si.on_update or [])
                    )
                new.append(inst)
            bb.instructions = new


WC_W = HID + L + TRANS + 1          # packed bf16 weight columns: atT | lw2 | w1T | w2T
FP_W = BS + 3                       # packed f32 row: cinv | (-b2, +b2) | 1.0


def _build(maxw_default=1):
    import concourse.bass as bass
    import concourse.mybir as mybir
    import concourse.tile as tile

    f32 = mybir.dt.float32
    bf16 = mybir.dt.bfloat16

    nc = bass.Bass("TRN2", target_bir_lowering=False, debug=False, num_devices=NCORES)

    xt_d = nc.declare_dram_parameter("xt", [HID, BS], bf16, isOutput=False)
    xn_d = nc.declare_dram_parameter("xn", [HID, BS], bf16, isOutput=False)
    mt_d = nc.declare_dram_parameter("mt", [L, BS], bf16, isOutput=False)
    wc_d = nc.declare_dram_parameter("wc", [128, WC_W], bf16, isOutput=False)
    fp_d = nc.declare_dram_parameter("fp", [1, FP_W], f32, isOutput=False)
    cb_d = nc.declare_dram_parameter("cbv", [1, BS], f32, isOutput=False)
    c0b1_d = nc.declare_dram_parameter("c0b1", [TRANS, 2], f32, isOutput=False)
    out_d = nc.declare_dram_parameter("out", [1, 1], f32, isOutput=True)

    AF = mybir.ActivationFunctionType
    ALU = mybir.AluOpType

    with tile.TileContext(nc) as tc:
        with (
            tc.tile_pool(name="const", bufs=1) as cpool,
            tc.tile_pool(name="xload", bufs=NT) as xpool,
            tc.tile_pool(name="mload", bufs=NT) as mpool,
            tc.tile_pool(name="work", bufs=2) as wpool,
            tc.tile_pool(name="psum_u", bufs=2, space="PSUM") as pu,
            tc.tile_pool(name="psum_v", bufs=2, space="PSUM") as pv,
            tc.tile_pool(name="psum_h2", bufs=2, space="PSUM") as ph2,
            tc.tile_pool(name="psum_e", bufs=2, space="PSUM") as pe,
        ):
            # ---- constants: 3 packed DMAs + 1 broadcast, all on the (idle)
            # gpsimd SWDGE queue so the Sync HWDGE ring is free for bulk data
            wc_sb = cpool.tile([128, WC_W], bf16, tag="wc")
            nc.gpsimd.dma_start(wc_sb[:], wc_d[:, :])
            fp_sb = cpool.tile([1, FP_W], f32, tag="fp")
            nc.gpsimd.dma_start(fp_sb[:], fp_d[:, :])
            c0b1_sb = cpool.tile([TRANS, 2], f32, tag="c0b1")
            nc.gpsimd.dma_start(c0b1_sb[:], c0b1_d[:, :])
            # 1/counts broadcast across all 128 partitions (DRAM-side step-0 AP)
            cb_sb = cpool.tile([128, BS], f32, tag="cb")
            nc.gpsimd.dma_start(cb_sb[:], cb_d.ap().broadcast_to([128, BS]))

            def atT(c):
                return wc_sb[:, c * TRANS : (c + 1) * TRANS]

            def lw2(c):
                return wc_sb[:, HID + c * TRANS : HID + (c + 1) * TRANS]

            w1T = wc_sb[:, HID + L : HID + L + TRANS]
            w2T = wc_sb[:, HID + L + TRANS : HID + L + TRANS + 1]
            c0 = c0b1_sb[:, 0:1]
            b1 = c0b1_sb[:, 1:2]

            # per-(tile, stream) softplus partial sums
            acc_sb = cpool.tile([1, 2 * NT], f32, tag="acc")
            nc.vector.memset(acc_sb[:, :], 0.0)

            # ---- bulk loads on the Sync HWDGE ring, tile-0 first ----
            mt_t, xt_t, xn_t = [], [], []
            for i in range(NT):
                sl = slice(i * BT, (i + 1) * BT)
                mt_i = mpool.tile([128, LC, BT], bf16, tag="mt")
                nc.sync.dma_start(
                    mt_i[:], mt_d[:, sl].rearrange("(c p) n -> p c n", p=128)
                )
                xt_i = xpool.tile([128, HC, BT], bf16, tag="xt")
                nc.sync.dma_start(
                    xt_i[:], xt_d[:, sl].rearrange("(c p) n -> p c n", p=128)
                )
                xn_i = xpool.tile([128, HC, BT], bf16, tag="xn")
                nc.sync.dma_start(
                    xn_i[:], xn_d[:, sl].rearrange("(c p) n -> p c n", p=128)
                )
                mt_t.append(mt_i); xt_t.append(xt_i); xn_t.append(xn_i)

            # ---- main loop over batch tiles ----
            for i in range(NT):
                sl = slice(i * BT, (i + 1) * BT)

                # v = (mask @ LW2).T for this tile, then vs = v / counts
                v_ps = pv.tile([128, BT], f32, tag="v")
                for c in range(LC):
                    nc.tensor.matmul(
                        v_ps[:, :],
                        lw2(c),
                        mt_t[i][:, c, :],
                        start=(c == 0),
                        stop=(c == LC - 1),
                    )
                vs_sb = wpool.tile([128, BT], f32, tag="vs")
                nc.vector.tensor_mul(vs_sb[:, :], v_ps[:, :], cb_sb[:, sl])

                for s, x_i in enumerate((xt_t[i], xn_t[i])):
                    u_ps = pu.tile([128, BT], f32, tag="u")
                    for c in range(HC):
                        nc.tensor.matmul(
                            u_ps[:, :],
                            atT(c),
                            x_i[:, c, :],
                            start=(c == 0),
                            stop=(c == HC - 1),
                        )
                    # u += c0 + vs in place (one DVE op), then relu -> bf16 on ACT
                    nc.vector.scalar_tensor_tensor(
                        u_ps[:, :],
                        u_ps[:, :],
                        c0,
                        vs_sb[:, :],
                        op0=ALU.add,
                        op1=ALU.add,
                    )
                    h1_sb = wpool.tile([128, BT], bf16, tag="h1")
                    nc.scalar.activation(h1_sb[:, :], u_ps[:, :], AF.Relu)

                    h2_ps = ph2.tile([128, BT], f32, tag="h2")
                    nc.tensor.matmul(
                        h2_ps[:, :], w1T, h1_sb[:, :], start=True, stop=True
                    )
                    h2_sb = wpool.tile([128, BT], bf16, tag="h2s")
                    nc.scalar.activation(h2_sb[:, :], h2_ps[:, :], AF.Relu, bias=b1)

                    e_ps = pe.tile([1, BT], f32, tag="e")
                    nc.tensor.matmul(
                        e_ps[:, :], w2T, h2_sb[:, :], start=True, stop=True
                    )
                    # joint stream (s==0): softplus(-(e+b2)); marginal: softplus(e+b2)
                    # softplus(x) = ln(1 + exp(x)) — this walrus ACT table has
                    # exp+ln+relu in one func set but no native softplus.
                    # |e+b2| stays O(1) here so exp cannot overflow.
                    sgn = -1.0 if s == 0 else 1.0
                    k = 2 * i + s
                    nc.scalar.activation(
                        e_ps[:, :],
                        e_ps[:, :],
                        AF.Exp,
                        bias=fp_sb[:, BS + s : BS + s + 1],
                        scale=sgn,
                    )
                    sp_sb = wpool.tile([1, BT], f32, tag="sp")
                    nc.scalar.activation(
                        sp_sb[:, :],
                        e_ps[:, :],
                        AF.Ln,
                        bias=fp_sb[:, BS + 2 : BS + 3],
                        accum_out=acc_sb[:, k : k + 1],
                    )

            res_sb = cpool.tile([1, 1], f32, tag="res")
            nc.vector.reduce_sum(res_sb[:, :], acc_sb[:, :], axis=mybir.AxisListType.X)
            nc.sync.dma_start(out_d[:, :], res_sb[:, :])

    _split_sync_waits(nc, mybir, maxw_default=maxw_default, maxw_drain=1)
    return nc


def _get_nc():
    if "nc" not in _CACHE:
        _CACHE["nc"] = _build()
    return _CACHE["nc"]


def _prep_inputs(text_embed, label_embed, target, perm,
                 W_text, b_text, W_label, b_label, W0, b0, W1, b1, W2, b2):
    f64 = np.float64
    W0t = W0[:, :TRANS].astype(f64)
    W0l = W0[:, TRANS:].astype(f64)
    A_t = W0t @ W_text.astype(f64)                                   # [T, HID]
    LW2 = (label_embed.astype(f64) @ W_label.T.astype(f64)) @ W0l.T  # [L, T]
    c0 = b0.astype(f64) + W0t @ b_text.astype(f64) + W0l @ b_label.astype(f64)

    # packed bf16 weights: [128, atT(768) | lw2(512) | w1T(128) | w2T(1)]
    atT_p = np.ascontiguousarray(A_t.T).reshape(HC, 128, TRANS).transpose(1, 0, 2).reshape(128, HID)
    lw2_p = np.ascontiguousarray(LW2).reshape(LC, 128, TRANS).transpose(1, 0, 2).reshape(128, L)
    wc = np.concatenate(
        [atT_p, lw2_p, W1.T.astype(np.float64), W2.T.reshape(TRANS, 1).astype(np.float64)],
        axis=1).astype(BF16)
    c0b1 = np.stack([c0, b1.astype(np.float64)], axis=1).astype(np.float32)
    b2val = float(np.asarray(b2).reshape(-1)[0])

    counts = np.maximum(target.sum(axis=1), 1).astype(np.float64)
    cinv = (1.0 / counts).astype(np.float32)                         # [B]

    text_T = np.ascontiguousarray(text_embed.T).astype(BF16)         # [HID, B]
    mask_T = np.ascontiguousarray(target.T.astype(np.float32)).astype(BF16)  # [L, B]
    perm = np.asarray(perm).astype(np.int64)

    in_maps = []
    for k in range(NCORES):
        sl = slice(k * BS, (k + 1) * BS)
        cinv_k = cinv[sl]
        fp = np.concatenate([cinv_k, [-b2val, b2val, 1.0]]).astype(np.float32).reshape(1, FP_W)
        in_maps.append({
            "xt": np.ascontiguousarray(text_T[:, sl]),
            "xn": np.ascontiguousarray(text_T[:, perm[sl]]),
            "mt": np.ascontiguousarray(mask_T[:, sl]),
            "wc": wc, "fp": fp,
            "cbv": cinv_k.reshape(1, BS).copy(),
            "c0b1": c0b1,
        })
    return in_maps, b2val

def _run(in_maps, b2val, trace=False):
    from concourse.bass_utils import run_bass_kernel_spmd

    nc = _get_nc()
    res = run_bass_kernel_spmd(nc, in_maps, list(range(NCORES)), trace=trace)
    total = sum(float(res.results[k]["out"][0, 0]) for k in range(NCORES))
    return np.float32(total / B), res


def kernel(text_embed, label_embed, target, perm,
           W_text, b_text, W_label, b_label, W0, b0, W1, b1, W2, b2):
    in_maps, b2val = _prep_inputs(
        text_embed, label_embed, target, perm,
        W_text, b_text, W_label, b_label, W0, b0, W1, b1, W2, b2)
    out, _ = _run(in_maps, b2val)
    return out
